# revision 38
# baseline (speedup 1.0000x reference)
"""MinibatchDiscrimination kernel for 8 Trainium2 NeuronCores.

Reference computation:
    m = (x @ T).reshape(B, K, D)            # B=512, F=512, K=32, D=16
    abs_diffs[i,j,k] = sum_d |m[i,k,d] - m[j,k,d]|
    feats[i,k] = sum_j exp(-abs_diffs[i,j,k])
    out = concat([x, feats], axis=1)

Sharding: batch rows over the 8 cores.  Each core receives x with rows
rotated so that its own 64 rows come first, runs an identical SPMD
program that computes the full GEMM (m is needed for all j anyway) and
feats for local rows i=0..63 against all 512 j.

Device pipeline per core:
  1. PE-transpose x -> xT, fp32 GEMM mT[kd, b] = T^T @ x^T (4 PSUM-resident
     kd-chunks of [128, 512]).
  2. For each (i, kd-chunk): one-instruction |mT - mT[:, i]| -> bf16 tile,
     issued on DVE (tensor_scalar sub+abs_max) or ACT (activation Abs with
     per-partition bias = -m_i) to balance the two engines.
  3. PE mask-matmul (block-diagonal 0/1 weights) reduces the 16 d's of each
     kernel k, accumulating L1 distances for 4 i's into one PSUM tile
     [128 = 4i x 32k, 512 j].
  4. One ACT Exp(scale=-1, accum_out) per 4-i group fuses exp and the j-sum,
     yielding feats columns.
Host concatenates [x, feats].
"""

import numpy as np
import ml_dtypes
from contextlib import ExitStack

import concourse.bass as bass
import concourse.tile as tile
from concourse import bacc
from concourse import mybir
from concourse.bass_utils import run_bass_kernel_spmd
B, F, K, D = 512, 512, 32, 16
KD = K * D            # 512
P = 128               # partitions
NCORES = 8
NI = B // NCORES      # 64 i's per core
NG = NI // 4          # 16 groups of 4 i's
NC_CHUNK = KD // P    # 4 kd chunks

F32 = mybir.dt.float32
BF16 = mybir.dt.bfloat16

USE_CUSTOM_ABS = True


def _register_abs_sub():
    """Register (once per process) a fused custom-DVE op |in0 - s0|.

    The stock DVE ALU-op vocabulary has no float absolute-value, so the
    stock path needs two instructions (subtract; max(t, -t)).  The
    custom-uop table gives us maxx(Src0 - C0, C0 - Src0) in a single
    1-elem/cycle/lane instruction.
    """
    from concourse import dve_ops
    from concourse.dve_spec import Spec, Src0, C0, maxx, lower
    from concourse.dve_uop import DveOpSpec

    name = "ABS_SUB_ANT"
    if name in dve_ops._SUB_OPCODE_FOR_NAME:
        return next(op for op in dve_ops.OPS if op.name == name)
    spec = Spec(body=maxx(Src0 - C0, C0 - Src0),
                reference=lambda in0, in1, s0, s1, imm2: np.abs(in0 - s0))
    opcode = dve_ops._CUSTOM_DVE_ROW_BASE + len(dve_ops.OPS)
    assert opcode < 0x20
    shas = {}
    for ver in ("v3", "v4"):
        try:
            s = DveOpSpec(name=name, opcode=opcode, uops=lower(spec, ver=ver),
                          rd1_en=False)
            shas[ver] = s.sha(ver)
        except Exception:
            pass
    op = dve_ops.DveOp(name, spec, subdim=False, uops_sha=shas)
    dve_ops.OPS.append(op)
    dve_ops.CUSTOM_DVE_SPECS[name] = spec
    dve_ops._SUB_OPCODE_FOR_NAME[name] = opcode
    return op


def _act_handles(i, c):
    """Which abs passes go to the Scalar engine (rest go to Vector).

    ACT does |x - m_i| in one activation(Abs, bias) pass (~624 ns); DVE
    does it in one fused custom-DVE op (~654 ns).  ~120/256 on ACT
    balances the engines once ACT's exp/copy overheads are added.
    """
    return c == 0 or (c == 1 and i % 8 != 0)


def emit_program(tc, feats_ap, xt_ap, t_ap, mask_ap, ni=NI, act_handles=_act_handles):
    """Emit the tile program.  feats_ap: [128, ni//4] f32 dram out.

    Wait-discipline: this walrus build rejects any instruction carrying
    more than ONE semaphore wait.  Hence: every tile a PE matmul reads
    is (re)written by a single engine (DVE for GEMM operands, ACT for
    the mask), and the first mask-matmul of every group consumes an
    ACT-produced abs tile so the one ACT wait also covers the PSUM-slot
    WAR on the previous exp.
    """
    nc = tc.nc
    ng = ni // 4
    abs_op = _register_abs_sub() if USE_CUSTOM_ABS else None
    with ExitStack() as ctx:
        const = ctx.enter_context(tc.tile_pool(name="const", bufs=1))

        # ---- load inputs (xt = x^T from host) ----
        # GEMM inputs are declared float32r end-to-end (DRAM + SBUF): the PE
        # streams fp32r rows at full speed (~1.5e-4 l2 rounding on m --
        # negligible vs the bf16 abs tiles), and no staging copies are needed.
        F32R = mybir.dt.float32r
        if xt_ap.dtype != F32R:
            xt_ap = xt_ap.bitcast(F32R)
        if t_ap.dtype != F32R:
            t_ap = t_ap.bitcast(F32R)
        xt2 = [const.tile([P, B], F32R, tag=f"xt2{f}", name=f"xt2{f}") for f in range(4)]
        tb2 = [const.tile([P, KD], F32R, tag=f"tb2{f}", name=f"tb2{f}") for f in range(4)]
        # f-interleaved so the first GEMM accumulation chain's operands
        # arrive as early as possible
        for f in range(4):
            nc.sync.dma_start(xt2[f][:], xt_ap[f * P:(f + 1) * P, :])
            nc.sync.dma_start(tb2[f][:], t_ap[f * P:(f + 1) * P, :])
        mask2 = const.tile([P, 4 * 32], BF16, tag="mask2")
        nc.sync.dma_start(mask2[:], mask_ap[:])

        # ---- GEMM: mT[c] [128 kd, 512 b] = sum_f T[f, c]^T @ x^T[f] ----
        # mTs (fp32, DVE scalar columns), mTb (bf16, DVE subtract input) and
        # negm (fp32, ACT bias) copies interleave with the GEMM chunks.
        mpsum = ctx.enter_context(tc.tile_pool(name="mpsum", bufs=1, space="PSUM"))
        mTp = [mpsum.tile([P, B], F32, tag=f"mTp{c}", name=f"mTp{c}") for c in range(4)]
        mTb = [const.tile([P, B], BF16, tag=f"mTb{c}", name=f"mTb{c}") for c in range(4)]
        # mTc holds the SAME bf16-rounded values as mTb, widened to fp32 (the
        # tensor_scalar scalar operand must be an fp32 AP).  Using consistent
        # rounding on both sides keeps the diagonal |m_i - m_i| exactly zero.
        mTc = [const.tile([P, B], F32, tag=f"mTc{c}", name=f"mTc{c}") for c in range(4)]
        negm = [const.tile([P, B], F32, tag=f"negm{c}", name=f"negm{c}") for c in range(4)]
        for c in range(4):
            for f in range(4):
                nc.tensor.matmul(
                    mTp[c][:], tb2[f][:, c * P:(c + 1) * P], xt2[f][:],
                    start=(f == 0), stop=(f == 3),
                )
            nc.vector.tensor_copy(mTb[c][:], mTp[c][:])
            nc.scalar.copy(mTc[c][:], mTb[c][:])
            nc.scalar.mul(negm[c][:], mTp[c][:], -1.0)

        feats_sb = const.tile([P, ng], F32, tag="feats")

        # Separate abs pools per producing engine: shared slots would create
        # cross-engine WAW deps -> 2 sem waits (walrus limit is 1).
        abs_act = ctx.enter_context(tc.tile_pool(name="absa", bufs=6))
        abs_dve = ctx.enter_context(tc.tile_pool(name="absd", bufs=10))
        diff_pool = ctx.enter_context(tc.tile_pool(name="diffp", bufs=4))
        spool = ctx.enter_context(tc.tile_pool(name="spool", bufs=3, space="PSUM"))
        epool = ctx.enter_context(tc.tile_pool(name="epool", bufs=2))

        def emit_abs(i, c):
            if act_handles(i, c):
                ab = abs_act.tile([P, B], BF16, tag="aba", name="aba")
                nc.scalar.activation(
                    ab[:], mTp[c][:], mybir.ActivationFunctionType.Abs,
                    bias=negm[c][:, i:i + 1], scale=1.0,
                )
            elif abs_op is not None:
                ab = abs_dve.tile([P, B], BF16, tag="abd", name="abd")
                nc.vector._custom_dve(
                    abs_op, out=ab[:], in0=mTb[c][:], s0=mTc[c][:, i:i + 1],
                )
            else:
                df = diff_pool.tile([P, B], BF16, tag="df", name="df")
                nc.vector.tensor_scalar_sub(
                    df[:], mTb[c][:], mTc[c][:, i:i + 1],
                )
                ab = abs_dve.tile([P, B], BF16, tag="abd", name="abd")
                nc.vector.scalar_tensor_tensor(
                    ab[:], df[:], -1.0, df[:],
                    mybir.AluOpType.mult, mybir.AluOpType.max,
                )
            return ab

        for g in range(ng):
            sp = spool.tile([P, B], F32, tag="sp")
            for q in range(4):
                for c in range(4):
                    ab = emit_abs(4 * g + q, c)
                    nc.tensor.matmul(
                        sp[32 * q:32 * q + 32, :],
                        mask2[:, 32 * c:32 * (c + 1)],
                        ab[:], start=(c == 0), stop=(c == 3),
                        tile_position=(0, 32 * q),
                    )
            ed = epool.tile([P, B], BF16, tag="ed")
            nc.scalar.activation(
                ed[:], sp[:], mybir.ActivationFunctionType.Exp,
                scale=-1.0, accum_out=feats_sb[:, g:g + 1],
            )

        nc.sync.dma_start(feats_ap[:], feats_sb[:, :])


def make_mask():
    """4 stacked [128, 32] weight slices, one per kd chunk c.

    Slice c maps kd-chunk partition p -> output row k = (128*c + p) // D.
    lhsT of an M=32 matmul; the 4-i packing into PSUM quarters is done
    via tile_position=(0, 32*q).
    """
    mask = np.zeros((P, 4 * 32), dtype=np.float32)
    for c in range(4):
        for p in range(P):
            k = (128 * c + p) // D
            mask[p, 32 * c + k] = 1.0
    return mask.astype(ml_dtypes.bfloat16)


def build_nc(ni=NI, act_handles=_act_handles):
    nc = bacc.Bacc("TRN2", target_bir_lowering=False, debug=False,
                   num_devices=NCORES)
    F32R = mybir.dt.float32r
    xt_ap = nc.dram_tensor("xt", [F, B], F32R, kind="ExternalInput").ap()
    t_ap = nc.dram_tensor("t", [F, KD], F32R, kind="ExternalInput").ap()
    mask_ap = nc.dram_tensor("mask", [P, 4 * 32], BF16, kind="ExternalInput").ap()
    feats_ap = nc.dram_tensor("feats", [P, ni // 4], F32,
                              kind="ExternalOutput").ap()
    with tile.TileContext(nc) as tc:
        emit_program(tc, feats_ap, xt_ap, t_ap, mask_ap, ni=ni,
                     act_handles=act_handles)
    nc.compile()
    return nc


def decode_feats(fs, ni=NI):
    """[128, ni//4] device layout -> [ni, K] feats."""
    fl = np.asarray(fs, dtype=np.float32).reshape(4, K, ni // 4)  # [q, k, g]
    return fl.transpose(2, 0, 1).reshape(ni, K)                   # [4g+q, k]


def run_on_hw(x, T, ni=NI, trace=False, act_handles=_act_handles):
    x = np.ascontiguousarray(np.asarray(x, dtype=np.float32))
    T = np.ascontiguousarray(np.asarray(T, dtype=np.float32))
    mask = make_mask()
    nc = build_nc(ni=ni, act_handles=act_handles)
    in_maps = [
        {"xt": np.ascontiguousarray(np.roll(x, -64 * c, axis=0).T),
         "t": T, "mask": mask}
        for c in range(NCORES)
    ]
    res = run_bass_kernel_spmd(nc, in_maps, list(range(NCORES)), trace=trace)
    feats = np.empty((B, K), dtype=np.float32)
    for c in range(NCORES):
        feats[64 * c:64 * (c + 1)] = decode_feats(res.results[c]["feats"], ni=ni)
    return feats, res


def kernel(x, T):
    feats, _ = run_on_hw(x, T)
    return np.concatenate([np.asarray(x, np.float32), feats], axis=1)


# revision 42
# speedup vs baseline: 1.0141x; 1.0141x over previous
"""MinibatchDiscrimination kernel for 8 Trainium2 NeuronCores.

Reference computation:
    m = (x @ T).reshape(B, K, D)            # B=512, F=512, K=32, D=16
    abs_diffs[i,j,k] = sum_d |m[i,k,d] - m[j,k,d]|
    feats[i,k] = sum_j exp(-abs_diffs[i,j,k])
    out = concat([x, feats], axis=1)

Sharding: batch rows over the 8 cores.  Each core receives x with rows
rotated so that its own 64 rows come first, runs an identical SPMD
program that computes the full GEMM (m is needed for all j anyway) and
feats for local rows i=0..63 against all 512 j.

Device pipeline per core:
  1. PE-transpose x -> xT, fp32 GEMM mT[kd, b] = T^T @ x^T (4 PSUM-resident
     kd-chunks of [128, 512]).
  2. For each (i, kd-chunk): one-instruction |mT - mT[:, i]| -> bf16 tile,
     issued on DVE (tensor_scalar sub+abs_max) or ACT (activation Abs with
     per-partition bias = -m_i) to balance the two engines.
  3. PE mask-matmul (block-diagonal 0/1 weights) reduces the 16 d's of each
     kernel k, accumulating L1 distances for 4 i's into one PSUM tile
     [128 = 4i x 32k, 512 j].
  4. One ACT Exp(scale=-1, accum_out) per 4-i group fuses exp and the j-sum,
     yielding feats columns.
Host concatenates [x, feats].
"""

import numpy as np
import ml_dtypes
from contextlib import ExitStack

import concourse.bass as bass
import concourse.tile as tile
from concourse import bacc
from concourse import mybir
from concourse.bass_utils import run_bass_kernel_spmd
B, F, K, D = 512, 512, 32, 16
KD = K * D            # 512
P = 128               # partitions
NCORES = 8
NI = B // NCORES      # 64 i's per core
NG = NI // 4          # 16 groups of 4 i's
NC_CHUNK = KD // P    # 4 kd chunks

F32 = mybir.dt.float32
BF16 = mybir.dt.bfloat16

USE_CUSTOM_ABS = True


def _register_abs_sub():
    """Register (once per process) a fused custom-DVE op |in0 - s0|.

    The stock DVE ALU-op vocabulary has no float absolute-value, so the
    stock path needs two instructions (subtract; max(t, -t)).  The
    custom-uop table gives us maxx(Src0 - C0, C0 - Src0) in a single
    1-elem/cycle/lane instruction.
    """
    from concourse import dve_ops
    from concourse.dve_spec import Spec, Src0, C0, maxx, lower
    from concourse.dve_uop import DveOpSpec

    name = "ABS_SUB_ANT"
    if name in dve_ops._SUB_OPCODE_FOR_NAME:
        return next(op for op in dve_ops.OPS if op.name == name)
    spec = Spec(body=maxx(Src0 - C0, C0 - Src0),
                reference=lambda in0, in1, s0, s1, imm2: np.abs(in0 - s0))
    opcode = dve_ops._CUSTOM_DVE_ROW_BASE + len(dve_ops.OPS)
    assert opcode < 0x20
    shas = {}
    for ver in ("v3", "v4"):
        try:
            s = DveOpSpec(name=name, opcode=opcode, uops=lower(spec, ver=ver),
                          rd1_en=False)
            shas[ver] = s.sha(ver)
        except Exception:
            pass
    op = dve_ops.DveOp(name, spec, subdim=False, uops_sha=shas)
    dve_ops.OPS.append(op)
    dve_ops.CUSTOM_DVE_SPECS[name] = spec
    dve_ops._SUB_OPCODE_FOR_NAME[name] = opcode
    return op


def _act_handles(i, c):
    """Which abs passes go to the Scalar engine (rest go to Vector).

    ACT does |x - m_i| in one activation(Abs, bias) pass (~624 ns); DVE
    does it in one fused custom-DVE op (~654 ns).  ~124/256 on ACT
    balances the engines once ACT's exp/copy overheads are added.
    """
    return c == 0 or (c == 1 and i % 16 != 0)


def emit_program(tc, feats_ap, xt_ap, t_ap, mask_ap, ni=NI, act_handles=_act_handles):
    """Emit the tile program.  feats_ap: [128, ni//4] f32 dram out.

    Wait-discipline: this walrus build rejects any instruction carrying
    more than ONE semaphore wait.  Hence: every tile a PE matmul reads
    is (re)written by a single engine (DVE for GEMM operands, ACT for
    the mask), and the first mask-matmul of every group consumes an
    ACT-produced abs tile so the one ACT wait also covers the PSUM-slot
    WAR on the previous exp.
    """
    nc = tc.nc
    ng = ni // 4
    abs_op = _register_abs_sub() if USE_CUSTOM_ABS else None
    with ExitStack() as ctx:
        const = ctx.enter_context(tc.tile_pool(name="const", bufs=1))

        # ---- load inputs (xt = x^T from host) ----
        F32R = mybir.dt.float32r
        if xt_ap.dtype == F32R:
            xt_ap = xt_ap.bitcast(F32)
        if t_ap.dtype == F32R:
            t_ap = t_ap.bitcast(F32)
        xtb = [const.tile([P, B], F32, tag=f"xtb{f}", name=f"xtb{f}") for f in range(4)]
        tb = [const.tile([P, KD], F32, tag=f"tb{f}", name=f"tb{f}") for f in range(4)]
        for f in range(4):
            nc.sync.dma_start(xtb[f][:], xt_ap[f * P:(f + 1) * P, :])
            nc.sync.dma_start(tb[f][:], t_ap[f * P:(f + 1) * P, :])
        maskt = const.tile([P, 4 * 32], BF16, tag="mask")
        nc.sync.dma_start(maskt[:], mask_ap[:])

        # ---- stage DMA'd tiles behind one engine sem each ----
        # GEMM operands through DVE, rounded to fp32r (full-speed PE rows,
        # ~1.5e-4 l2 rounding on m -- negligible vs the bf16 abs tiles);
        # mask through ACT.  Staging measured ~1.5% faster end-to-end than
        # DMAing straight into f32r tiles (tighter PE scheduling).
        xt2 = [const.tile([P, B], F32R, tag=f"xt2{f}", name=f"xt2{f}") for f in range(4)]
        tb2 = [const.tile([P, KD], F32R, tag=f"tb2{f}", name=f"tb2{f}") for f in range(4)]
        for f in range(4):
            nc.vector.tensor_copy(tb2[f][:], tb[f][:])
            nc.vector.tensor_copy(xt2[f][:], xtb[f][:])
        mask2 = const.tile([P, 4 * 32], BF16, tag="mask2")
        nc.scalar.copy(mask2[:], maskt[:])

        # ---- GEMM: mT[c] [128 kd, 512 b] = sum_f T[f, c]^T @ x^T[f] ----
        # mTs (fp32, DVE scalar columns), mTb (bf16, DVE subtract input) and
        # negm (fp32, ACT bias) copies interleave with the GEMM chunks.
        mpsum = ctx.enter_context(tc.tile_pool(name="mpsum", bufs=1, space="PSUM"))
        mTp = [mpsum.tile([P, B], F32, tag=f"mTp{c}", name=f"mTp{c}") for c in range(4)]
        mTb = [const.tile([P, B], BF16, tag=f"mTb{c}", name=f"mTb{c}") for c in range(4)]
        # mTc holds the SAME bf16-rounded values as mTb, widened to fp32 (the
        # tensor_scalar scalar operand must be an fp32 AP).  Using consistent
        # rounding on both sides keeps the diagonal |m_i - m_i| exactly zero.
        mTc = [const.tile([P, B], F32, tag=f"mTc{c}", name=f"mTc{c}") for c in range(4)]
        negm = [const.tile([P, B], F32, tag=f"negm{c}", name=f"negm{c}") for c in range(4)]
        for c in range(4):
            for f in range(4):
                nc.tensor.matmul(
                    mTp[c][:], tb2[f][:, c * P:(c + 1) * P], xt2[f][:],
                    start=(f == 0), stop=(f == 3),
                )
            nc.vector.tensor_copy(mTb[c][:], mTp[c][:])
            nc.vector.tensor_copy(mTc[c][:], mTb[c][:])
            nc.scalar.mul(negm[c][:], mTp[c][:], -1.0)

        feats_sb = const.tile([P, ng], F32, tag="feats")

        # Separate abs pools per producing engine: shared slots would create
        # cross-engine WAW deps -> 2 sem waits (walrus limit is 1).
        abs_act = ctx.enter_context(tc.tile_pool(name="absa", bufs=6))
        abs_dve = ctx.enter_context(tc.tile_pool(name="absd", bufs=10))
        diff_pool = ctx.enter_context(tc.tile_pool(name="diffp", bufs=4))
        spool = ctx.enter_context(tc.tile_pool(name="spool", bufs=3, space="PSUM"))
        epool = ctx.enter_context(tc.tile_pool(name="epool", bufs=2))

        def emit_abs(i, c):
            if act_handles(i, c):
                ab = abs_act.tile([P, B], BF16, tag="aba", name="aba")
                nc.scalar.activation(
                    ab[:], mTp[c][:], mybir.ActivationFunctionType.Abs,
                    bias=negm[c][:, i:i + 1], scale=1.0,
                )
            elif abs_op is not None:
                ab = abs_dve.tile([P, B], BF16, tag="abd", name="abd")
                nc.vector._custom_dve(
                    abs_op, out=ab[:], in0=mTb[c][:], s0=mTc[c][:, i:i + 1],
                )
            else:
                df = diff_pool.tile([P, B], BF16, tag="df", name="df")
                nc.vector.tensor_scalar_sub(
                    df[:], mTb[c][:], mTc[c][:, i:i + 1],
                )
                ab = abs_dve.tile([P, B], BF16, tag="abd", name="abd")
                nc.vector.scalar_tensor_tensor(
                    ab[:], df[:], -1.0, df[:],
                    mybir.AluOpType.mult, mybir.AluOpType.max,
                )
            return ab

        for g in range(ng):
            sp = spool.tile([P, B], F32, tag="sp")
            for q in range(4):
                for c in range(4):
                    ab = emit_abs(4 * g + q, c)
                    nc.tensor.matmul(
                        sp[32 * q:32 * q + 32, :],
                        mask2[:, 32 * c:32 * (c + 1)],
                        ab[:], start=(c == 0), stop=(c == 3),
                        tile_position=(0, 32 * q),
                    )
            ed = epool.tile([P, B], BF16, tag="ed")
            nc.scalar.activation(
                ed[:], sp[:], mybir.ActivationFunctionType.Exp,
                scale=-1.0, accum_out=feats_sb[:, g:g + 1],
            )

        nc.sync.dma_start(feats_ap[:], feats_sb[:, :])


def make_mask():
    """4 stacked [128, 32] weight slices, one per kd chunk c.

    Slice c maps kd-chunk partition p -> output row k = (128*c + p) // D.
    lhsT of an M=32 matmul; the 4-i packing into PSUM quarters is done
    via tile_position=(0, 32*q).
    """
    mask = np.zeros((P, 4 * 32), dtype=np.float32)
    for c in range(4):
        for p in range(P):
            k = (128 * c + p) // D
            mask[p, 32 * c + k] = 1.0
    return mask.astype(ml_dtypes.bfloat16)


def build_nc(ni=NI, act_handles=_act_handles):
    nc = bacc.Bacc("TRN2", target_bir_lowering=False, debug=False,
                   num_devices=NCORES)
    xt_ap = nc.dram_tensor("xt", [F, B], F32, kind="ExternalInput").ap()
    t_ap = nc.dram_tensor("t", [F, KD], F32, kind="ExternalInput").ap()
    mask_ap = nc.dram_tensor("mask", [P, 4 * 32], BF16, kind="ExternalInput").ap()
    feats_ap = nc.dram_tensor("feats", [P, ni // 4], F32,
                              kind="ExternalOutput").ap()
    with tile.TileContext(nc) as tc:
        emit_program(tc, feats_ap, xt_ap, t_ap, mask_ap, ni=ni,
                     act_handles=act_handles)
    nc.compile()
    return nc


def decode_feats(fs, ni=NI):
    """[128, ni//4] device layout -> [ni, K] feats."""
    fl = np.asarray(fs, dtype=np.float32).reshape(4, K, ni // 4)  # [q, k, g]
    return fl.transpose(2, 0, 1).reshape(ni, K)                   # [4g+q, k]


def run_on_hw(x, T, ni=NI, trace=False, act_handles=_act_handles):
    x = np.ascontiguousarray(np.asarray(x, dtype=np.float32))
    T = np.ascontiguousarray(np.asarray(T, dtype=np.float32))
    mask = make_mask()
    nc = build_nc(ni=ni, act_handles=act_handles)
    in_maps = [
        {"xt": np.ascontiguousarray(np.roll(x, -64 * c, axis=0).T),
         "t": T, "mask": mask}
        for c in range(NCORES)
    ]
    res = run_bass_kernel_spmd(nc, in_maps, list(range(NCORES)), trace=trace)
    feats = np.empty((B, K), dtype=np.float32)
    for c in range(NCORES):
        feats[64 * c:64 * (c + 1)] = decode_feats(res.results[c]["feats"], ni=ni)
    return feats, res


def kernel(x, T):
    feats, _ = run_on_hw(x, T)
    return np.concatenate([np.asarray(x, np.float32), feats], axis=1)


# revision 45
# speedup vs baseline: 1.0141x; 1.0000x over previous
"""MinibatchDiscrimination kernel for 8 Trainium2 NeuronCores.

Reference computation:
    m = (x @ T).reshape(B, K, D)            # B=512, F=512, K=32, D=16
    abs_diffs[i,j,k] = sum_d |m[i,k,d] - m[j,k,d]|
    feats[i,k] = sum_j exp(-abs_diffs[i,j,k])
    out = concat([x, feats], axis=1)

Sharding: batch rows over the 8 cores.  Each core receives x with rows
rotated so that its own 64 rows come first, runs an identical SPMD
program that computes the full GEMM (m is needed for all j anyway) and
feats for local rows i=0..63 against all 512 j.

Device pipeline per core:
  1. fp32r GEMM mT[kd, b] = T^T @ x^T (x^T comes pre-transposed from the
     host; 4 PSUM-resident kd-chunks of [128, 512]).
  2. For each (i, kd-chunk): one-instruction |mT - mT[:, i]| -> bf16 tile,
     issued on DVE (runtime-registered custom fused op) or ACT (activation
     Abs with per-partition bias = -m_i) to balance the two engines.
  3. PE mask-matmul (block-diagonal 0/1 weights) reduces the 16 d's of each
     kernel k, accumulating L1 distances for 4 i's into one PSUM tile
     [128 = 4i x 32k, 512 j].
  4. One ACT Exp(scale=-1, accum_out) per 4-i group fuses exp and the j-sum,
     yielding feats columns.
Host concatenates [x, feats].
"""

import numpy as np
import ml_dtypes
from contextlib import ExitStack

import concourse.bass as bass
import concourse.tile as tile
from concourse import bacc
from concourse import mybir
from concourse.bass_utils import run_bass_kernel_spmd
B, F, K, D = 512, 512, 32, 16
KD = K * D            # 512
P = 128               # partitions
NCORES = 8
NI = B // NCORES      # 64 i's per core
NG = NI // 4          # 16 groups of 4 i's
NC_CHUNK = KD // P    # 4 kd chunks

F32 = mybir.dt.float32
BF16 = mybir.dt.bfloat16

USE_CUSTOM_ABS = True


def _register_abs_sub():
    """Register (once per process) a fused custom-DVE op |in0 - s0|.

    The stock DVE ALU-op vocabulary has no float absolute-value, so the
    stock path needs two instructions (subtract; max(t, -t)).  The
    custom-uop table gives us maxx(Src0 - C0, C0 - Src0) in a single
    1-elem/cycle/lane instruction.
    """
    from concourse import dve_ops
    from concourse.dve_spec import Spec, Src0, C0, maxx, lower
    from concourse.dve_uop import DveOpSpec

    name = "ABS_SUB_ANT"
    if name in dve_ops._SUB_OPCODE_FOR_NAME:
        return next(op for op in dve_ops.OPS if op.name == name)
    spec = Spec(body=maxx(Src0 - C0, C0 - Src0),
                reference=lambda in0, in1, s0, s1, imm2: np.abs(in0 - s0))
    opcode = dve_ops._CUSTOM_DVE_ROW_BASE + len(dve_ops.OPS)
    assert opcode < 0x20
    shas = {}
    for ver in ("v3", "v4"):
        try:
            s = DveOpSpec(name=name, opcode=opcode, uops=lower(spec, ver=ver),
                          rd1_en=False)
            shas[ver] = s.sha(ver)
        except Exception:
            pass
    op = dve_ops.DveOp(name, spec, subdim=False, uops_sha=shas)
    dve_ops.OPS.append(op)
    dve_ops.CUSTOM_DVE_SPECS[name] = spec
    dve_ops._SUB_OPCODE_FOR_NAME[name] = opcode
    return op


def _act_handles(i, c):
    """Which abs passes go to the Scalar engine (rest go to Vector).

    ACT does |x - m_i| in one activation(Abs, bias) pass (~624 ns); DVE
    does it in one fused custom-DVE op (~654 ns).  ~124/256 on ACT
    balances the engines once ACT's exp/copy overheads are added.
    """
    return c == 0 or (c == 1 and i % 16 != 0)


def emit_program(tc, feats_ap, xt_ap, t_ap, mask_ap, ni=NI, act_handles=_act_handles):
    """Emit the tile program.  feats_ap: [128, ni//4] f32 dram out.

    Wait-discipline: this walrus build rejects any instruction carrying
    more than ONE semaphore wait.  Hence: every tile a PE matmul reads
    is (re)written by a single engine (DVE for GEMM operands, ACT for
    the mask), and the first mask-matmul of every group consumes an
    ACT-produced abs tile so the one ACT wait also covers the PSUM-slot
    WAR on the previous exp.
    """
    nc = tc.nc
    ng = ni // 4
    abs_op = _register_abs_sub() if USE_CUSTOM_ABS else None
    with ExitStack() as ctx:
        const = ctx.enter_context(tc.tile_pool(name="const", bufs=1))

        # ---- load inputs (xt = x^T from host) ----
        F32R = mybir.dt.float32r
        if xt_ap.dtype == F32R:
            xt_ap = xt_ap.bitcast(F32)
        if t_ap.dtype == F32R:
            t_ap = t_ap.bitcast(F32)
        xtb = [const.tile([P, B], F32, tag=f"xtb{f}", name=f"xtb{f}") for f in range(4)]
        tb = [const.tile([P, KD], F32, tag=f"tb{f}", name=f"tb{f}") for f in range(4)]
        for f in range(4):
            nc.sync.dma_start(xtb[f][:], xt_ap[f * P:(f + 1) * P, :])
            nc.sync.dma_start(tb[f][:], t_ap[f * P:(f + 1) * P, :])
        maskt = const.tile([P, 4 * 32], BF16, tag="mask")
        nc.sync.dma_start(maskt[:], mask_ap[:])

        # ---- stage DMA'd tiles behind one engine sem each ----
        # GEMM operands through DVE, rounded to fp32r (full-speed PE rows,
        # ~1.5e-4 l2 rounding on m -- negligible vs the bf16 abs tiles);
        # mask through ACT.  Staging measured ~1.5% faster end-to-end than
        # DMAing straight into f32r tiles (tighter PE scheduling).
        xt2 = [const.tile([P, B], F32R, tag=f"xt2{f}", name=f"xt2{f}") for f in range(4)]
        tb2 = [const.tile([P, KD], F32R, tag=f"tb2{f}", name=f"tb2{f}") for f in range(4)]
        for f in range(4):
            nc.vector.tensor_copy(tb2[f][:], tb[f][:])
            nc.vector.tensor_copy(xt2[f][:], xtb[f][:])
        mask2 = const.tile([P, 4 * 32], BF16, tag="mask2")
        nc.scalar.copy(mask2[:], maskt[:])

        # ---- GEMM: mT[c] [128 kd, 512 b] = sum_f T[f, c]^T @ x^T[f] ----
        # mTs (fp32, DVE scalar columns), mTb (bf16, DVE subtract input) and
        # negm (fp32, ACT bias) copies interleave with the GEMM chunks.
        mpsum = ctx.enter_context(tc.tile_pool(name="mpsum", bufs=1, space="PSUM"))
        mTp = [mpsum.tile([P, B], F32, tag=f"mTp{c}", name=f"mTp{c}") for c in range(4)]
        mTb = [const.tile([P, B], BF16, tag=f"mTb{c}", name=f"mTb{c}") for c in range(4)]
        # mTc holds the SAME bf16-rounded values as mTb, widened to fp32 (the
        # tensor_scalar scalar operand must be an fp32 AP).  Using consistent
        # rounding on both sides keeps the diagonal |m_i - m_i| exactly zero.
        mTc = [const.tile([P, B], F32, tag=f"mTc{c}", name=f"mTc{c}") for c in range(4)]
        negm = [const.tile([P, B], F32, tag=f"negm{c}", name=f"negm{c}") for c in range(4)]
        for c in range(4):
            for f in range(4):
                nc.tensor.matmul(
                    mTp[c][:], tb2[f][:, c * P:(c + 1) * P], xt2[f][:],
                    start=(f == 0), stop=(f == 3),
                )
            nc.vector.tensor_copy(mTb[c][:], mTp[c][:])
            nc.vector.tensor_copy(mTc[c][:], mTb[c][:])
            nc.scalar.mul(negm[c][:], mTp[c][:], -1.0)

        feats_sb = const.tile([P, ng], F32, tag="feats")

        # Separate abs pools per producing engine: shared slots would create
        # cross-engine WAW deps -> 2 sem waits (walrus limit is 1).
        abs_act = ctx.enter_context(tc.tile_pool(name="absa", bufs=6))
        abs_dve = ctx.enter_context(tc.tile_pool(name="absd", bufs=10))
        diff_pool = ctx.enter_context(tc.tile_pool(name="diffp", bufs=4))
        spool = ctx.enter_context(tc.tile_pool(name="spool", bufs=3, space="PSUM"))
        epool = ctx.enter_context(tc.tile_pool(name="epool", bufs=2))

        def emit_abs(i, c):
            if act_handles(i, c):
                ab = abs_act.tile([P, B], BF16, tag="aba", name="aba")
                nc.scalar.activation(
                    ab[:], mTp[c][:], mybir.ActivationFunctionType.Abs,
                    bias=negm[c][:, i:i + 1], scale=1.0,
                )
            elif abs_op is not None:
                ab = abs_dve.tile([P, B], BF16, tag="abd", name="abd")
                nc.vector._custom_dve(
                    abs_op, out=ab[:], in0=mTb[c][:], s0=mTc[c][:, i:i + 1],
                )
            else:
                df = diff_pool.tile([P, B], BF16, tag="df", name="df")
                nc.vector.tensor_scalar_sub(
                    df[:], mTb[c][:], mTc[c][:, i:i + 1],
                )
                ab = abs_dve.tile([P, B], BF16, tag="abd", name="abd")
                nc.vector.scalar_tensor_tensor(
                    ab[:], df[:], -1.0, df[:],
                    mybir.AluOpType.mult, mybir.AluOpType.max,
                )
            return ab

        for g in range(ng):
            sp = spool.tile([P, B], F32, tag="sp")
            for q in range(4):
                for c in range(4):
                    ab = emit_abs(4 * g + q, c)
                    nc.tensor.matmul(
                        sp[32 * q:32 * q + 32, :],
                        mask2[:, 32 * c:32 * (c + 1)],
                        ab[:], start=(c == 0), stop=(c == 3),
                        tile_position=(0, 32 * q),
                    )
            ed = epool.tile([P, B], BF16, tag="ed")
            nc.scalar.activation(
                ed[:], sp[:], mybir.ActivationFunctionType.Exp,
                scale=-1.0, accum_out=feats_sb[:, g:g + 1],
            )

        nc.sync.dma_start(feats_ap[:], feats_sb[:, :])


def make_mask():
    """4 stacked [128, 32] weight slices, one per kd chunk c.

    Slice c maps kd-chunk partition p -> output row k = (128*c + p) // D.
    lhsT of an M=32 matmul; the 4-i packing into PSUM quarters is done
    via tile_position=(0, 32*q).
    """
    mask = np.zeros((P, 4 * 32), dtype=np.float32)
    for c in range(4):
        for p in range(P):
            k = (128 * c + p) // D
            mask[p, 32 * c + k] = 1.0
    return mask.astype(ml_dtypes.bfloat16)


def build_nc(ni=NI, act_handles=_act_handles):
    nc = bacc.Bacc("TRN2", target_bir_lowering=False, debug=False,
                   num_devices=NCORES)
    xt_ap = nc.dram_tensor("xt", [F, B], F32, kind="ExternalInput").ap()
    t_ap = nc.dram_tensor("t", [F, KD], F32, kind="ExternalInput").ap()
    mask_ap = nc.dram_tensor("mask", [P, 4 * 32], BF16, kind="ExternalInput").ap()
    feats_ap = nc.dram_tensor("feats", [P, ni // 4], F32,
                              kind="ExternalOutput").ap()
    with tile.TileContext(nc) as tc:
        emit_program(tc, feats_ap, xt_ap, t_ap, mask_ap, ni=ni,
                     act_handles=act_handles)
    nc.compile()
    return nc


def decode_feats(fs, ni=NI):
    """[128, ni//4] device layout -> [ni, K] feats."""
    fl = np.asarray(fs, dtype=np.float32).reshape(4, K, ni // 4)  # [q, k, g]
    return fl.transpose(2, 0, 1).reshape(ni, K)                   # [4g+q, k]


def run_on_hw(x, T, ni=NI, trace=False, act_handles=_act_handles):
    x = np.ascontiguousarray(np.asarray(x, dtype=np.float32))
    T = np.ascontiguousarray(np.asarray(T, dtype=np.float32))
    mask = make_mask()
    nc = build_nc(ni=ni, act_handles=act_handles)
    in_maps = [
        {"xt": np.ascontiguousarray(np.roll(x, -64 * c, axis=0).T),
         "t": T, "mask": mask}
        for c in range(NCORES)
    ]
    res = run_bass_kernel_spmd(nc, in_maps, list(range(NCORES)), trace=trace)
    feats = np.empty((B, K), dtype=np.float32)
    for c in range(NCORES):
        feats[64 * c:64 * (c + 1)] = decode_feats(res.results[c]["feats"], ni=ni)
    return feats, res


def kernel(x, T):
    feats, _ = run_on_hw(x, T)
    return np.concatenate([np.asarray(x, np.float32), feats], axis=1)
